# revision 29
# baseline (speedup 1.0000x reference)
"""Causal single-head attention (projections + softmax(QK^T)V) on 8 TRN2 cores.

Sharding: pure data parallelism over the batch dim (B=8 -> one batch element
per NeuronCore). Each core runs an identical Bass/Tile program on its shard.

Per-core dataflow (everything transposed so no on-chip transposes of the big
score matrix are needed):
  - inputs are fed pre-transposed/fp16 from the host as xT [E,S] (laid out
    [ei, j, eo, s] so every DMA run is 8KB contiguous per partition) so the
    E-contraction sits on the SBUF partition dim,
  - qT/kT/vT [d=128, S] = W_xT.T @ xT accumulated in PSUM fp32, bias added
    on the VectorE eviction to SBUF fp16,
  - scores^T block [k,q] = kT_blk.T @ qT, exp on ScalarE with fused 1/sqrt(d)
    scale (no max-subtraction: |scores/sqrt(d)| <= ~4 so exp cannot overflow),
  - causal mask applied only on diagonal 128x128 blocks (entries with q < k
    in lower tiles are never read by the AV stage),
  - out[q, dv] and the softmax denominator come from one PE accumulation:
    [num | den] = exp_blk.T @ [v | 1]; normalize on VectorE; DMA out fp32.

Matmuls run in fp16 (1 cyc/row on PE vs 4 for fp32) with fp32 PSUM
accumulation and an fp32 softmax; measured end-to-end scale-relative error
vs the fp32 reference is ~3e-4.
"""

import math

import numpy as np

import concourse.bass as bass  # noqa: F401  (registers AP machinery)
import concourse.tile as tile
from concourse import bacc, mybir
from concourse.bass_utils import run_bass_kernel_spmd

B, S, E = 8, 2048, 1024
DQ, DV = 128, 128
P = 128
EO = E // P          # 8 e-chunks
ST = S // P          # 16 sequence tiles of 128
NCH = 512            # psum free-dim chunk (one bank of fp32)
NJ = S // NCH        # 4 s-chunks
NCORES = 8
SCALE = 1.0 / math.sqrt(DQ)

f16 = mybir.dt.float16
f32 = mybir.dt.float32

_CACHE = {}
LAST_RESULT = None  # BassKernelResults of the most recent run (for profiling)


def _build_nc():
    nc = bacc.Bacc("TRN2", target_bir_lowering=False, debug=False)

    qx_e = nc.declare_dram_parameter("qx", [P, NJ, EO, NCH], f16, isOutput=False)
    kx_e = nc.declare_dram_parameter("kx", [P, NJ, EO, NCH], f16, isOutput=False)
    vx_e = nc.declare_dram_parameter("vx", [P, NJ, EO, NCH], f16, isOutput=False)
    wq_e = nc.declare_dram_parameter("wq", [P, EO, DQ], f16, isOutput=False)
    wk_e = nc.declare_dram_parameter("wk", [P, EO, DQ], f16, isOutput=False)
    wv_e = nc.declare_dram_parameter("wv", [P, EO, DV], f16, isOutput=False)
    bq_e = nc.declare_dram_parameter("bq", [P, 1], f32, isOutput=False)
    bk_e = nc.declare_dram_parameter("bk", [P, 1], f32, isOutput=False)
    bv_e = nc.declare_dram_parameter("bv", [P, 1], f32, isOutput=False)
    mask_e = nc.declare_dram_parameter("mask", [P, P], f16, isOutput=False)
    id_e = nc.declare_dram_parameter("ident", [P, P], f16, isOutput=False)
    out_e = nc.declare_dram_parameter("out", [S, DV], f32, isOutput=True)

    Exp = mybir.ActivationFunctionType.Exp

    with (
        tile.TileContext(nc) as tc,
        tc.tile_pool(name="consts", bufs=1) as consts,
        tc.tile_pool(name="inx", bufs=5) as inx,
        tc.tile_pool(name="acts", bufs=1) as acts,
        tc.tile_pool(name="outp", bufs=10) as outp,
        tc.tile_pool(name="pp", bufs=2, space="PSUM") as pp,
        tc.tile_pool(name="ps_s", bufs=2, space="PSUM") as ps_s_pool,
        tc.tile_pool(name="ps_n", bufs=2, space="PSUM") as ps_n_pool,
    ):
        # weights on the sync HWDGE ahead of the streamed input chunks; the
        # small per-partition consts go on the gpsimd SWDGE so their issue
        # latency doesn't delay the input stream.
        # ident first on the scalar HWDGE: it gates the PE warm-up burst
        id_sb = consts.tile([P, P], f16, tag="ident")
        nc.scalar.dma_start(id_sb[:], id_e.ap())
        w_sb = {}
        for nm, ext in (("wq", wq_e), ("wk", wk_e), ("wv", wv_e)):
            t = consts.tile([P, EO, DQ], f16, tag=nm)
            nc.scalar.dma_start(t[:], ext.ap())
            w_sb[nm] = t
        mask_sb = consts.tile([P, P], f16, tag="mask")
        nc.scalar.dma_start(mask_sb[:], mask_e.ap())
        b_sb = {}
        for nm, ext in (("bq", bq_e), ("bk", bk_e), ("bv", bv_e)):
            t = consts.tile([P, 1], f32, tag=nm)
            nc.gpsimd.dma_start(t[:], ext.ap())
            b_sb[nm] = t

        qT = acts.tile([P, S], f16, tag="qT")
        kT = acts.tile([P, S], f16, tag="kT")
        vT = acts.tile([P, S], f16, tag="vT")
        v_ext = acts.tile([P, ST, DV + 1], f16, tag="vex")
        nc.vector.memset(v_ext[:, :, DV : DV + 1], 1.0)
        E_big = acts.tile([P, ST, S], f16, tag="exp")

        proj_specs = (
            (qx_e, w_sb["wq"], b_sb["bq"], qT),
            (kx_e, w_sb["wk"], b_sb["bk"], kT),
            (vx_e, w_sb["wv"], b_sb["bv"], vT),
        )

        # PE warm-up while the first input chunk is in flight (~5us): engages
        # the HAM clock gate (cold PE runs at 1.2 GHz instead of 2.4) and
        # keeps it engaged until real work arrives.  The operand is memset
        # on-chip so the burst depends on no DMA; one junk DVE copy consumes
        # the result so DCE keeps it.
        wu_in = consts.tile([P, NCH], f16, tag="wu_in")
        nc.vector.memset(wu_in[:], 1.0)
        wu_ps = pp.tile([P, NCH], f32, tag="pp")
        junk = consts.tile([P, P], f32, tag="junk")
        for _ in range(22):
            nc.tensor.matmul(wu_ps[:], wu_in[:, :P], wu_in[:], start=True, stop=True)
        nc.vector.tensor_copy(junk[:], wu_ps[:, :P])

        deferred_out = []
        for j in range(NJ):
            with nc.named_scope(f"round{j}"):
                sl = slice(j * NCH, (j + 1) * NCH)
                # projections for this s-chunk: q, k, v
                for ti, (xe, wt, bt, dst) in enumerate(proj_specs):
                    xc = inx.tile([P, EO, NCH], f16, tag="inx")
                    nc.sync.dma_start(xc[:], xe.ap()[:, j])
                    ps = pp.tile([P, NCH], f32, tag="pp")
                    for eo in range(EO):
                        nc.tensor.matmul(
                            ps[:],
                            wt[:, eo, :],
                            xc[:, eo, :],
                            start=(eo == 0),
                            stop=(eo == EO - 1),
                        )
                    nc.vector.tensor_scalar_add(dst[:, sl], ps[:], bt[:])


                # v blocks [s, dv] for the 4 new sequence tiles
                for st in range(4 * j, 4 * j + 4):
                    tp = ps_n_pool.tile([P, P], f16, tag="ps_n")
                    nc.tensor.transpose(tp[:], vT[:, st * P : (st + 1) * P], id_sb[:])
                    nc.vector.tensor_copy(v_ext[:, st, 0:DV], tp[:])

                # scores^T for q-chunk j against all causal k tiles.  fp16
                # PSUM output packs two k-tiles into one bank so one exp call
                # covers the pair (fewer ACT per-instruction overheads).
                for kt in range(0, 4 * j + 4, 2):
                    ps = ps_s_pool.tile([P, 2, NCH], f32, tag="ps_s")
                    for u in range(2):
                        nc.tensor.matmul(
                            ps[:, u, :],
                            kT[:, (kt + u) * P : (kt + u + 1) * P],
                            qT[:, sl],
                            start=True,
                            stop=True,
                        )
                    nc.scalar.activation(
                        E_big[:, kt : kt + 2, sl], ps[:], Exp, scale=SCALE
                    )

                # mask the 4 new diagonal blocks
                for kt in range(4 * j, 4 * j + 4):
                    d0 = kt * P
                    nc.vector.tensor_mul(
                        E_big[:, kt, d0 : d0 + P],
                        E_big[:, kt, d0 : d0 + P],
                        mask_sb[:],
                    )

                # AV + normalize + store for the 4 new q tiles
                for qt in range(4 * j, 4 * j + 4):
                    pn = ps_n_pool.tile([P, DV + 1], f32, tag="ps_n")
                    for kt in range(qt + 1):
                        nc.tensor.matmul(
                            pn[:],
                            E_big[:, kt, qt * P : (qt + 1) * P],
                            v_ext[:, kt, :],
                            start=(kt == 0),
                            stop=(kt == qt),
                        )
                    rec = outp.tile([P, 1], f32, tag="rec")
                    nc.vector.reciprocal(rec[:], pn[:, DV : DV + 1])
                    ot = outp.tile([P, DV], f32, tag="out")
                    nc.vector.tensor_scalar_mul(ot[:], pn[:, 0:DV], rec[:])
                    nc.gpsimd.dma_start(out_e.ap()[qt * P : (qt + 1) * P, :], ot[:])

    nc.compile()
    return nc


def _get_nc():
    if "nc" not in _CACHE:
        _CACHE["nc"] = _build_nc()
    return _CACHE["nc"]


def _prep_consts(Wq, bq, Wk, bk, Wv, bv):
    def prep_w(W):  # [D, E] f32 -> W.T [E, D] -> [ei, eo, D] fp16
        WT = W.T.astype(np.float16)  # [E, D]
        return np.ascontiguousarray(WT.reshape(EO, P, -1).transpose(1, 0, 2))

    consts = {
        "wq": prep_w(Wq),
        "wk": prep_w(Wk),
        "wv": prep_w(Wv),
        "bq": np.ascontiguousarray(bq.astype(np.float32)[:, None]),
        "bk": np.ascontiguousarray(bk.astype(np.float32)[:, None]),
        "bv": np.ascontiguousarray(bv.astype(np.float32)[:, None]),
        "mask": np.triu(np.ones((P, P), np.float16)),
        "ident": np.eye(P, dtype=np.float16),
    }
    return consts


def _prep_x(x):  # [S, E] f32 -> xT [E, S] fp16 -> [ei, j, eo, s_in_chunk]
    xT = x.astype(np.float16).T  # [E, S]
    x4 = xT.reshape(EO, P, NJ, NCH)  # [eo, ei, j, s]
    return np.ascontiguousarray(x4.transpose(1, 2, 0, 3))


def kernel(query, key_in, value, Wq, bq, Wk, bk, Wv, bv):
    global LAST_RESULT
    query = np.asarray(query, dtype=np.float32)
    key_in = np.asarray(key_in, dtype=np.float32)
    value = np.asarray(value, dtype=np.float32)
    consts = _prep_consts(
        np.asarray(Wq), np.asarray(bq), np.asarray(Wk),
        np.asarray(bk), np.asarray(Wv), np.asarray(bv),
    )
    in_maps = []
    for b in range(NCORES):
        m = dict(consts)
        m["qx"] = _prep_x(query[b])
        m["kx"] = _prep_x(key_in[b])
        m["vx"] = _prep_x(value[b])
        in_maps.append(m)

    nc = _get_nc()
    res = run_bass_kernel_spmd(nc, in_maps, core_ids=list(range(NCORES)))
    LAST_RESULT = res
    return np.stack([res.results[i]["out"] for i in range(NCORES)], axis=0)


# revision 31
# speedup vs baseline: 1.1620x; 1.1620x over previous
"""Causal single-head attention (projections + softmax(QK^T)V) on 8 TRN2 cores.

Sharding: pure data parallelism over the batch dim (B=8 -> one batch element
per NeuronCore). Each core runs an identical Bass/Tile program on its shard.

Per-core dataflow (everything transposed so no on-chip transposes of the big
score matrix are needed):
  - inputs are fed pre-transposed/fp16 from the host as xT [E,S] (laid out
    [ei, j, eo, s] so every DMA run is 8KB contiguous per partition) so the
    E-contraction sits on the SBUF partition dim,
  - qT/kT/vT [d=128, S] = W_xT.T @ xT accumulated in PSUM fp32, bias added
    on the VectorE eviction to SBUF fp16,
  - scores^T block [k,q] = kT_blk.T @ qT, exp on ScalarE with fused 1/sqrt(d)
    scale (no max-subtraction: |scores/sqrt(d)| <= ~4 so exp cannot overflow),
  - causal mask applied only on diagonal 128x128 blocks (entries with q < k
    in lower tiles are never read by the AV stage),
  - out[q, dv] and the softmax denominator come from one PE accumulation:
    [num | den] = exp_blk.T @ [v | 1]; normalize on VectorE; DMA out fp32.

Matmuls run in fp16 (1 cyc/row on PE vs 4 for fp32) with fp32 PSUM
accumulation and an fp32 softmax; measured end-to-end scale-relative error
vs the fp32 reference is ~3e-4.
"""

import math

import numpy as np

import concourse.bass as bass  # noqa: F401  (registers AP machinery)
import concourse.tile as tile
from concourse import bacc, mybir
from concourse.bass_utils import run_bass_kernel_spmd

B, S, E = 8, 2048, 1024
DQ, DV = 128, 128
P = 128
EO = E // P          # 8 e-chunks
ST = S // P          # 16 sequence tiles of 128
NCH = 512            # psum free-dim chunk (one bank of fp32)
NJ = S // NCH        # 4 s-chunks
NCORES = 8
SCALE = 1.0 / math.sqrt(DQ)

f16 = mybir.dt.float16
f32 = mybir.dt.float32

_CACHE = {}
LAST_RESULT = None  # BassKernelResults of the most recent run (for profiling)


def _build_nc():
    nc = bacc.Bacc("TRN2", target_bir_lowering=False, debug=False)

    qx_e = nc.declare_dram_parameter("qx", [P, NJ, EO, NCH], f16, isOutput=False)
    kx_e = nc.declare_dram_parameter("kx", [P, NJ, EO, NCH], f16, isOutput=False)
    vx_e = nc.declare_dram_parameter("vx", [P, NJ, EO, NCH], f16, isOutput=False)
    wq_e = nc.declare_dram_parameter("wq", [P, EO, DQ], f16, isOutput=False)
    wk_e = nc.declare_dram_parameter("wk", [P, EO, DQ], f16, isOutput=False)
    wv_e = nc.declare_dram_parameter("wv", [P, EO, DV], f16, isOutput=False)
    bq_e = nc.declare_dram_parameter("bq", [P, 1], f32, isOutput=False)
    bk_e = nc.declare_dram_parameter("bk", [P, 1], f32, isOutput=False)
    bv_e = nc.declare_dram_parameter("bv", [P, 1], f32, isOutput=False)
    mask_e = nc.declare_dram_parameter("mask", [P, P], f16, isOutput=False)
    id_e = nc.declare_dram_parameter("ident", [P, P], f16, isOutput=False)
    out_e = nc.declare_dram_parameter("out", [S, DV], f32, isOutput=True)

    Exp = mybir.ActivationFunctionType.Exp

    with (
        tile.TileContext(nc) as tc,
        tc.tile_pool(name="consts", bufs=1) as consts,
        tc.tile_pool(name="inx", bufs=6) as inx,
        tc.tile_pool(name="acts", bufs=1) as acts,
        tc.tile_pool(name="outp", bufs=10) as outp,
        tc.tile_pool(name="pp", bufs=2, space="PSUM") as pp,
        tc.tile_pool(name="ps_s", bufs=2, space="PSUM") as ps_s_pool,
        tc.tile_pool(name="ps_n", bufs=2, space="PSUM") as ps_n_pool,
    ):
        # weights on the sync HWDGE ahead of the streamed input chunks; the
        # small per-partition consts go on the gpsimd SWDGE so their issue
        # latency doesn't delay the input stream.
        # ident first on the scalar HWDGE: it gates the PE warm-up burst
        id_sb = consts.tile([P, P], f16, tag="ident")
        nc.scalar.dma_start(id_sb[:], id_e.ap())
        w_sb = {}
        for nm, ext in (("wq", wq_e), ("wk", wk_e), ("wv", wv_e)):
            t = consts.tile([P, EO, DQ], f16, tag=nm)
            nc.scalar.dma_start(t[:], ext.ap())
            w_sb[nm] = t
        mask_sb = consts.tile([P, P], f16, tag="mask")
        nc.scalar.dma_start(mask_sb[:], mask_e.ap())
        b_sb = {}
        for nm, ext in (("bq", bq_e), ("bk", bk_e), ("bv", bv_e)):
            t = consts.tile([P, 1], f32, tag=nm)
            nc.gpsimd.dma_start(t[:], ext.ap())
            b_sb[nm] = t

        qT = acts.tile([P, S], f16, tag="qT")
        kT = acts.tile([P, S], f16, tag="kT")
        vT = acts.tile([P, S], f16, tag="vT")
        v_ext = acts.tile([P, ST, DV + 1], f16, tag="vex")
        nc.vector.memset(v_ext[:, :, DV : DV + 1], 1.0)
        E_big = acts.tile([P, ST, S], f16, tag="exp")

        proj_specs = (
            (qx_e, w_sb["wq"], b_sb["bq"], qT),
            (kx_e, w_sb["wk"], b_sb["bk"], kT),
            (vx_e, w_sb["wv"], b_sb["bv"], vT),
        )

        deferred_out = []
        for j in range(NJ):
            with nc.named_scope(f"round{j}"):
                sl = slice(j * NCH, (j + 1) * NCH)
                # projections for this s-chunk: q, k, v
                for ti, (xe, wt, bt, dst) in enumerate(proj_specs):
                    xc = inx.tile([P, EO, NCH], f16, tag="inx")
                    nc.sync.dma_start(xc[:], xe.ap()[:, j])
                    ps = pp.tile([P, NCH], f32, tag="pp")
                    for eo in range(EO):
                        nc.tensor.matmul(
                            ps[:],
                            wt[:, eo, :],
                            xc[:, eo, :],
                            start=(eo == 0),
                            stop=(eo == EO - 1),
                        )
                    nc.vector.tensor_scalar_add(dst[:, sl], ps[:], bt[:])


                # v blocks [s, dv] for the 4 new sequence tiles
                for st in range(4 * j, 4 * j + 4):
                    tp = ps_n_pool.tile([P, P], f16, tag="ps_n")
                    nc.tensor.transpose(tp[:], vT[:, st * P : (st + 1) * P], id_sb[:])
                    nc.vector.tensor_copy(v_ext[:, st, 0:DV], tp[:])

                # scores^T for q-chunk j against all causal k tiles.  fp16
                # PSUM output packs two k-tiles into one bank so one exp call
                # covers the pair (fewer ACT per-instruction overheads).
                for kt in range(0, 4 * j + 4, 2):
                    ps = ps_s_pool.tile([P, 2, NCH], f32, tag="ps_s")
                    for u in range(2):
                        nc.tensor.matmul(
                            ps[:, u, :],
                            kT[:, (kt + u) * P : (kt + u + 1) * P],
                            qT[:, sl],
                            start=True,
                            stop=True,
                        )
                    nc.scalar.activation(
                        E_big[:, kt : kt + 2, sl], ps[:], Exp, scale=SCALE
                    )

                # mask the 4 new diagonal blocks
                for kt in range(4 * j, 4 * j + 4):
                    d0 = kt * P
                    nc.vector.tensor_mul(
                        E_big[:, kt, d0 : d0 + P],
                        E_big[:, kt, d0 : d0 + P],
                        mask_sb[:],
                    )

                # AV + normalize + store for the 4 new q tiles
                for qt in range(4 * j, 4 * j + 4):
                    pn = ps_n_pool.tile([P, DV + 1], f32, tag="ps_n")
                    for kt in range(qt + 1):
                        nc.tensor.matmul(
                            pn[:],
                            E_big[:, kt, qt * P : (qt + 1) * P],
                            v_ext[:, kt, :],
                            start=(kt == 0),
                            stop=(kt == qt),
                        )
                    rec = outp.tile([P, 1], f32, tag="rec")
                    nc.vector.reciprocal(rec[:], pn[:, DV : DV + 1])
                    ot = outp.tile([P, DV], f32, tag="out")
                    nc.vector.tensor_scalar_mul(ot[:], pn[:, 0:DV], rec[:])
                    nc.gpsimd.dma_start(out_e.ap()[qt * P : (qt + 1) * P, :], ot[:])

    nc.compile()
    return nc


def _get_nc():
    if "nc" not in _CACHE:
        _CACHE["nc"] = _build_nc()
    return _CACHE["nc"]


def _prep_consts(Wq, bq, Wk, bk, Wv, bv):
    def prep_w(W):  # [D, E] f32 -> W.T [E, D] -> [ei, eo, D] fp16
        WT = W.T.astype(np.float16)  # [E, D]
        return np.ascontiguousarray(WT.reshape(EO, P, -1).transpose(1, 0, 2))

    consts = {
        "wq": prep_w(Wq),
        "wk": prep_w(Wk),
        "wv": prep_w(Wv),
        "bq": np.ascontiguousarray(bq.astype(np.float32)[:, None]),
        "bk": np.ascontiguousarray(bk.astype(np.float32)[:, None]),
        "bv": np.ascontiguousarray(bv.astype(np.float32)[:, None]),
        "mask": np.triu(np.ones((P, P), np.float16)),
        "ident": np.eye(P, dtype=np.float16),
    }
    return consts


def _prep_x(x):  # [S, E] f32 -> xT [E, S] fp16 -> [ei, j, eo, s_in_chunk]
    xT = x.astype(np.float16).T  # [E, S]
    x4 = xT.reshape(EO, P, NJ, NCH)  # [eo, ei, j, s]
    return np.ascontiguousarray(x4.transpose(1, 2, 0, 3))


def kernel(query, key_in, value, Wq, bq, Wk, bk, Wv, bv):
    global LAST_RESULT
    query = np.asarray(query, dtype=np.float32)
    key_in = np.asarray(key_in, dtype=np.float32)
    value = np.asarray(value, dtype=np.float32)
    consts = _prep_consts(
        np.asarray(Wq), np.asarray(bq), np.asarray(Wk),
        np.asarray(bk), np.asarray(Wv), np.asarray(bv),
    )
    in_maps = []
    for b in range(NCORES):
        m = dict(consts)
        m["qx"] = _prep_x(query[b])
        m["kx"] = _prep_x(key_in[b])
        m["vx"] = _prep_x(value[b])
        in_maps.append(m)

    nc = _get_nc()
    res = run_bass_kernel_spmd(nc, in_maps, core_ids=list(range(NCORES)))
    LAST_RESULT = res
    return np.stack([res.results[i]["out"] for i in range(NCORES)], axis=0)


# revision 32
# speedup vs baseline: 1.1880x; 1.0224x over previous
"""Causal single-head attention (projections + softmax(QK^T)V) on 8 TRN2 cores.

Sharding: pure data parallelism over the batch dim (B=8 -> one batch element
per NeuronCore). Each core runs an identical Bass/Tile program on its shard.

Per-core dataflow (everything transposed so no on-chip transposes of the big
score matrix are needed):
  - inputs are fed pre-transposed/fp16 from the host as xT [E,S] (laid out
    [ei, j, eo, s] so every DMA run is 8KB contiguous per partition) so the
    E-contraction sits on the SBUF partition dim,
  - qT/kT/vT [d=128, S] = W_xT.T @ xT accumulated in PSUM fp32, bias added
    on the VectorE eviction to SBUF fp16,
  - scores^T block [k,q] = kT_blk.T @ qT, exp on ScalarE with fused 1/sqrt(d)
    scale (no max-subtraction: |scores/sqrt(d)| <= ~4 so exp cannot overflow),
  - causal mask applied only on diagonal 128x128 blocks (entries with q < k
    in lower tiles are never read by the AV stage),
  - out[q, dv] and the softmax denominator come from one PE accumulation:
    [num | den] = exp_blk.T @ [v | 1]; normalize on VectorE; DMA out fp32.

Matmuls run in fp16 (1 cyc/row on PE vs 4 for fp32) with fp32 PSUM
accumulation and an fp32 softmax; measured end-to-end scale-relative error
vs the fp32 reference is ~3e-4.
"""

import math

import numpy as np

import concourse.bass as bass  # noqa: F401  (registers AP machinery)
import concourse.tile as tile
from concourse import bacc, mybir
from concourse.bass_utils import run_bass_kernel_spmd

B, S, E = 8, 2048, 1024
DQ, DV = 128, 128
P = 128
EO = E // P          # 8 e-chunks
ST = S // P          # 16 sequence tiles of 128
NCH = 256            # s-chunk width per pipeline round
NJ = S // NCH        # 8 s-chunks
TPR = ST // NJ       # sequence tiles per round (2)
NCORES = 8
SCALE = 1.0 / math.sqrt(DQ)

f16 = mybir.dt.float16
f32 = mybir.dt.float32

_CACHE = {}
LAST_RESULT = None  # BassKernelResults of the most recent run (for profiling)


def _build_nc():
    nc = bacc.Bacc("TRN2", target_bir_lowering=False, debug=False)

    qx_e = nc.declare_dram_parameter("qx", [P, NJ, EO, NCH], f16, isOutput=False)
    kx_e = nc.declare_dram_parameter("kx", [P, NJ, EO, NCH], f16, isOutput=False)
    vx_e = nc.declare_dram_parameter("vx", [P, NJ, EO, NCH], f16, isOutput=False)
    wq_e = nc.declare_dram_parameter("wq", [P, EO, DQ], f16, isOutput=False)
    wk_e = nc.declare_dram_parameter("wk", [P, EO, DQ], f16, isOutput=False)
    wv_e = nc.declare_dram_parameter("wv", [P, EO, DV], f16, isOutput=False)
    bq_e = nc.declare_dram_parameter("bq", [P, 1], f32, isOutput=False)
    bk_e = nc.declare_dram_parameter("bk", [P, 1], f32, isOutput=False)
    bv_e = nc.declare_dram_parameter("bv", [P, 1], f32, isOutput=False)
    mask_e = nc.declare_dram_parameter("mask", [P, P], f16, isOutput=False)
    id_e = nc.declare_dram_parameter("ident", [P, P], f16, isOutput=False)
    out_e = nc.declare_dram_parameter("out", [S, DV], f32, isOutput=True)

    Exp = mybir.ActivationFunctionType.Exp

    with (
        tile.TileContext(nc) as tc,
        tc.tile_pool(name="consts", bufs=1) as consts,
        tc.tile_pool(name="inx", bufs=6) as inx,
        tc.tile_pool(name="acts", bufs=1) as acts,
        tc.tile_pool(name="outp", bufs=10) as outp,
        tc.tile_pool(name="pp", bufs=2, space="PSUM") as pp,
        tc.tile_pool(name="ps_s", bufs=4, space="PSUM") as ps_s_pool,
        tc.tile_pool(name="ps_n", bufs=2, space="PSUM") as ps_n_pool,
    ):
        # weights on the sync HWDGE ahead of the streamed input chunks; the
        # small per-partition consts go on the gpsimd SWDGE so their issue
        # latency doesn't delay the input stream.
        # ident first on the scalar HWDGE: it gates the PE warm-up burst
        id_sb = consts.tile([P, P], f16, tag="ident")
        nc.scalar.dma_start(id_sb[:], id_e.ap())
        w_sb = {}
        for nm, ext in (("wq", wq_e), ("wk", wk_e), ("wv", wv_e)):
            t = consts.tile([P, EO, DQ], f16, tag=nm)
            nc.scalar.dma_start(t[:], ext.ap())
            w_sb[nm] = t
        mask_sb = consts.tile([P, P], f16, tag="mask")
        nc.scalar.dma_start(mask_sb[:], mask_e.ap())
        b_sb = {}
        for nm, ext in (("bq", bq_e), ("bk", bk_e), ("bv", bv_e)):
            t = consts.tile([P, 1], f32, tag=nm)
            nc.gpsimd.dma_start(t[:], ext.ap())
            b_sb[nm] = t

        qT = acts.tile([P, S], f16, tag="qT")
        kT = acts.tile([P, S], f16, tag="kT")
        vT = acts.tile([P, S], f16, tag="vT")
        v_ext = acts.tile([P, ST, DV + 1], f16, tag="vex")
        nc.vector.memset(v_ext[:, :, DV : DV + 1], 1.0)
        E_big = acts.tile([P, ST, S], f16, tag="exp")

        proj_specs = (
            (qx_e, w_sb["wq"], b_sb["bq"], qT),
            (kx_e, w_sb["wk"], b_sb["bk"], kT),
            (vx_e, w_sb["wv"], b_sb["bv"], vT),
        )

        deferred_out = []
        for j in range(NJ):
            with nc.named_scope(f"round{j}"):
                sl = slice(j * NCH, (j + 1) * NCH)
                # projections for this s-chunk: q, k, v
                for ti, (xe, wt, bt, dst) in enumerate(proj_specs):
                    xc = inx.tile([P, EO, NCH], f16, tag="inx")
                    nc.sync.dma_start(xc[:], xe.ap()[:, j])
                    ps = pp.tile([P, NCH], f32, tag="pp")
                    for eo in range(EO):
                        nc.tensor.matmul(
                            ps[:],
                            wt[:, eo, :],
                            xc[:, eo, :],
                            start=(eo == 0),
                            stop=(eo == EO - 1),
                        )
                    nc.vector.tensor_scalar_add(dst[:, sl], ps[:], bt[:])


                # v blocks [s, dv] for the 4 new sequence tiles
                for st in range(TPR * j, TPR * (j + 1)):
                    tp = ps_n_pool.tile([P, P], f16, tag="ps_n")
                    nc.tensor.transpose(tp[:], vT[:, st * P : (st + 1) * P], id_sb[:])
                    nc.vector.tensor_copy(v_ext[:, st, 0:DV], tp[:])

                # scores^T for q-chunk j against all causal k tiles.  fp16
                # PSUM output packs two k-tiles into one bank so one exp call
                # covers the pair (fewer ACT per-instruction overheads).
                for kt in range(0, TPR * (j + 1), 2):
                    ps = ps_s_pool.tile([P, 2, NCH], f32, tag="ps_s")
                    for u in range(2):
                        nc.tensor.matmul(
                            ps[:, u, :],
                            kT[:, (kt + u) * P : (kt + u + 1) * P],
                            qT[:, sl],
                            start=True,
                            stop=True,
                        )
                    nc.scalar.activation(
                        E_big[:, kt : kt + 2, sl], ps[:], Exp, scale=SCALE
                    )

                # mask the 4 new diagonal blocks
                for kt in range(TPR * j, TPR * (j + 1)):
                    d0 = kt * P
                    nc.vector.tensor_mul(
                        E_big[:, kt, d0 : d0 + P],
                        E_big[:, kt, d0 : d0 + P],
                        mask_sb[:],
                    )

                # AV + normalize + store for the 4 new q tiles
                for qt in range(TPR * j, TPR * (j + 1)):
                    pn = ps_n_pool.tile([P, DV + 1], f32, tag="ps_n")
                    for kt in range(qt + 1):
                        nc.tensor.matmul(
                            pn[:],
                            E_big[:, kt, qt * P : (qt + 1) * P],
                            v_ext[:, kt, :],
                            start=(kt == 0),
                            stop=(kt == qt),
                        )
                    rec = outp.tile([P, 1], f32, tag="rec")
                    nc.vector.reciprocal(rec[:], pn[:, DV : DV + 1])
                    ot = outp.tile([P, DV], f32, tag="out")
                    nc.vector.tensor_scalar_mul(ot[:], pn[:, 0:DV], rec[:])
                    nc.gpsimd.dma_start(out_e.ap()[qt * P : (qt + 1) * P, :], ot[:])

    nc.compile()
    return nc


def _get_nc():
    if "nc" not in _CACHE:
        _CACHE["nc"] = _build_nc()
    return _CACHE["nc"]


def _prep_consts(Wq, bq, Wk, bk, Wv, bv):
    def prep_w(W):  # [D, E] f32 -> W.T [E, D] -> [ei, eo, D] fp16
        WT = W.T.astype(np.float16)  # [E, D]
        return np.ascontiguousarray(WT.reshape(EO, P, -1).transpose(1, 0, 2))

    consts = {
        "wq": prep_w(Wq),
        "wk": prep_w(Wk),
        "wv": prep_w(Wv),
        "bq": np.ascontiguousarray(bq.astype(np.float32)[:, None]),
        "bk": np.ascontiguousarray(bk.astype(np.float32)[:, None]),
        "bv": np.ascontiguousarray(bv.astype(np.float32)[:, None]),
        "mask": np.triu(np.ones((P, P), np.float16)),
        "ident": np.eye(P, dtype=np.float16),
    }
    return consts


def _prep_x(x):  # [S, E] f32 -> xT [E, S] fp16 -> [ei, j, eo, s_in_chunk]
    xT = x.astype(np.float16).T  # [E, S]
    x4 = xT.reshape(EO, P, NJ, NCH)  # [eo, ei, j, s]
    return np.ascontiguousarray(x4.transpose(1, 2, 0, 3))


def kernel(query, key_in, value, Wq, bq, Wk, bk, Wv, bv):
    global LAST_RESULT
    query = np.asarray(query, dtype=np.float32)
    key_in = np.asarray(key_in, dtype=np.float32)
    value = np.asarray(value, dtype=np.float32)
    consts = _prep_consts(
        np.asarray(Wq), np.asarray(bq), np.asarray(Wk),
        np.asarray(bk), np.asarray(Wv), np.asarray(bv),
    )
    in_maps = []
    for b in range(NCORES):
        m = dict(consts)
        m["qx"] = _prep_x(query[b])
        m["kx"] = _prep_x(key_in[b])
        m["vx"] = _prep_x(value[b])
        in_maps.append(m)

    nc = _get_nc()
    res = run_bass_kernel_spmd(nc, in_maps, core_ids=list(range(NCORES)))
    LAST_RESULT = res
    return np.stack([res.results[i]["out"] for i in range(NCORES)], axis=0)


# revision 33
# speedup vs baseline: 1.1959x; 1.0066x over previous
"""Causal single-head attention (projections + softmax(QK^T)V) on 8 TRN2 cores.

Sharding: pure data parallelism over the batch dim (B=8 -> one batch element
per NeuronCore). Each core runs an identical Bass/Tile program on its shard.

Per-core dataflow (everything transposed so no on-chip transposes of the big
score matrix are needed):
  - inputs are fed pre-transposed/fp16 from the host as xT [E,S] (laid out
    [ei, j, eo, s] so every DMA run is 8KB contiguous per partition) so the
    E-contraction sits on the SBUF partition dim,
  - qT/kT/vT [d=128, S] = W_xT.T @ xT accumulated in PSUM fp32, bias added
    on the VectorE eviction to SBUF fp16,
  - scores^T block [k,q] = kT_blk.T @ qT, exp on ScalarE with fused 1/sqrt(d)
    scale (no max-subtraction: |scores/sqrt(d)| <= ~4 so exp cannot overflow),
  - causal mask applied only on diagonal 128x128 blocks (entries with q < k
    in lower tiles are never read by the AV stage),
  - out[q, dv] and the softmax denominator come from one PE accumulation:
    [num | den] = exp_blk.T @ [v | 1]; normalize on VectorE; DMA out fp32.

Matmuls run in fp16 (1 cyc/row on PE vs 4 for fp32) with fp32 PSUM
accumulation and an fp32 softmax; measured end-to-end scale-relative error
vs the fp32 reference is ~3e-4.
"""

import math

import numpy as np

import concourse.bass as bass  # noqa: F401  (registers AP machinery)
import concourse.tile as tile
from concourse import bacc, mybir
from concourse.bass_utils import run_bass_kernel_spmd

B, S, E = 8, 2048, 1024
DQ, DV = 128, 128
P = 128
EO = E // P          # 8 e-chunks
ST = S // P          # 16 sequence tiles of 128
NCH = 256            # s-chunk width per pipeline round
NJ = S // NCH        # 8 s-chunks
TPR = ST // NJ       # sequence tiles per round (2)
NCORES = 8
SCALE = 1.0 / math.sqrt(DQ)

f16 = mybir.dt.float16
f32 = mybir.dt.float32

_CACHE = {}
LAST_RESULT = None  # BassKernelResults of the most recent run (for profiling)


def _build_nc():
    nc = bacc.Bacc("TRN2", target_bir_lowering=False, debug=False)

    qx_e = nc.declare_dram_parameter("qx", [P, NJ, EO, NCH], f16, isOutput=False)
    kx_e = nc.declare_dram_parameter("kx", [P, NJ, EO, NCH], f16, isOutput=False)
    vx_e = nc.declare_dram_parameter("vx", [P, NJ, EO, NCH], f16, isOutput=False)
    wq_e = nc.declare_dram_parameter("wq", [P, EO, DQ], f16, isOutput=False)
    wk_e = nc.declare_dram_parameter("wk", [P, EO, DQ], f16, isOutput=False)
    wv_e = nc.declare_dram_parameter("wv", [P, EO, DV], f16, isOutput=False)
    bq_e = nc.declare_dram_parameter("bq", [P, 1], f32, isOutput=False)
    bk_e = nc.declare_dram_parameter("bk", [P, 1], f32, isOutput=False)
    bv_e = nc.declare_dram_parameter("bv", [P, 1], f32, isOutput=False)
    mask_e = nc.declare_dram_parameter("mask", [P, P], f16, isOutput=False)
    id_e = nc.declare_dram_parameter("ident", [P, P], f16, isOutput=False)
    out_e = nc.declare_dram_parameter("out", [S, DV], f32, isOutput=True)

    Exp = mybir.ActivationFunctionType.Exp

    with (
        tile.TileContext(nc) as tc,
        tc.tile_pool(name="consts", bufs=1) as consts,
        tc.tile_pool(name="inx", bufs=6) as inx,
        tc.tile_pool(name="acts", bufs=1) as acts,
        tc.tile_pool(name="outp", bufs=10) as outp,
        tc.tile_pool(name="pp", bufs=2, space="PSUM") as pp,
        tc.tile_pool(name="ps_s", bufs=4, space="PSUM") as ps_s_pool,
        tc.tile_pool(name="ps_n", bufs=2, space="PSUM") as ps_n_pool,
    ):
        # Consts ride the scalar HWDGE / gpsimd SWDGE so the sync HWDGE ring
        # carries nothing but the streamed input chunks (no head-of-line waits).
        id_sb = consts.tile([P, P], f16, tag="ident")
        nc.scalar.dma_start(id_sb[:], id_e.ap())
        w_sb = {}
        for nm, ext in (("wq", wq_e), ("wk", wk_e), ("wv", wv_e)):
            t = consts.tile([P, EO, DQ], f16, tag=nm)
            nc.scalar.dma_start(t[:], ext.ap())
            w_sb[nm] = t
        mask_sb = consts.tile([P, P], f16, tag="mask")
        nc.scalar.dma_start(mask_sb[:], mask_e.ap())
        b_sb = {}
        for nm, ext in (("bq", bq_e), ("bk", bk_e), ("bv", bv_e)):
            t = consts.tile([P, 1], f32, tag=nm)
            nc.gpsimd.dma_start(t[:], ext.ap())
            b_sb[nm] = t

        qT = acts.tile([P, S], f16, tag="qT")
        kT = acts.tile([P, S], f16, tag="kT")
        vT = acts.tile([P, S], f16, tag="vT")
        v_ext = acts.tile([P, ST, DV + 1], f16, tag="vex")
        nc.vector.memset(v_ext[:, :, DV : DV + 1], 1.0)
        E_big = acts.tile([P, ST, S], f16, tag="exp")

        proj_specs = (
            (qx_e, w_sb["wq"], b_sb["bq"], qT),
            (kx_e, w_sb["wk"], b_sb["bk"], kT),
            (vx_e, w_sb["wv"], b_sb["bv"], vT),
        )

        for j in range(NJ):
            with nc.named_scope(f"round{j}"):
                sl = slice(j * NCH, (j + 1) * NCH)
                # projections for this s-chunk: q, k, v
                for ti, (xe, wt, bt, dst) in enumerate(proj_specs):
                    xc = inx.tile([P, EO, NCH], f16, tag="inx")
                    nc.sync.dma_start(xc[:], xe.ap()[:, j])
                    ps = pp.tile([P, NCH], f32, tag="pp")
                    for eo in range(EO):
                        nc.tensor.matmul(
                            ps[:],
                            wt[:, eo, :],
                            xc[:, eo, :],
                            start=(eo == 0),
                            stop=(eo == EO - 1),
                        )
                    nc.vector.tensor_scalar_add(dst[:, sl], ps[:], bt[:])


                # v blocks [s, dv] for this round's sequence tiles
                for st in range(TPR * j, TPR * (j + 1)):
                    tp = ps_n_pool.tile([P, P], f16, tag="ps_n")
                    nc.tensor.transpose(tp[:], vT[:, st * P : (st + 1) * P], id_sb[:])
                    nc.vector.tensor_copy(v_ext[:, st, 0:DV], tp[:])

                # scores^T for q-chunk j against all causal k tiles; two
                # k-tiles share one PSUM pair-tile so a single exp call covers
                # both (amortizes ACT per-instruction overhead).
                for kt in range(0, TPR * (j + 1), 2):
                    ps = ps_s_pool.tile([P, 2, NCH], f32, tag="ps_s")
                    for u in range(2):
                        nc.tensor.matmul(
                            ps[:, u, :],
                            kT[:, (kt + u) * P : (kt + u + 1) * P],
                            qT[:, sl],
                            start=True,
                            stop=True,
                        )
                    nc.scalar.activation(
                        E_big[:, kt : kt + 2, sl], ps[:], Exp, scale=SCALE
                    )

                # mask this round's diagonal blocks
                for kt in range(TPR * j, TPR * (j + 1)):
                    d0 = kt * P
                    nc.vector.tensor_mul(
                        E_big[:, kt, d0 : d0 + P],
                        E_big[:, kt, d0 : d0 + P],
                        mask_sb[:],
                    )

                # AV + normalize + store for this round's q tiles
                for qt in range(TPR * j, TPR * (j + 1)):
                    pn = ps_n_pool.tile([P, DV + 1], f32, tag="ps_n")
                    for kt in range(qt + 1):
                        nc.tensor.matmul(
                            pn[:],
                            E_big[:, kt, qt * P : (qt + 1) * P],
                            v_ext[:, kt, :],
                            start=(kt == 0),
                            stop=(kt == qt),
                        )
                    rec = outp.tile([P, 1], f32, tag="rec")
                    nc.vector.reciprocal(rec[:], pn[:, DV : DV + 1])
                    ot = outp.tile([P, DV], f32, tag="out")
                    nc.vector.tensor_scalar_mul(ot[:], pn[:, 0:DV], rec[:])
                    nc.gpsimd.dma_start(out_e.ap()[qt * P : (qt + 1) * P, :], ot[:])

    nc.compile()
    return nc


def _get_nc():
    if "nc" not in _CACHE:
        _CACHE["nc"] = _build_nc()
    return _CACHE["nc"]


def _prep_consts(Wq, bq, Wk, bk, Wv, bv):
    def prep_w(W):  # [D, E] f32 -> W.T [E, D] -> [ei, eo, D] fp16
        WT = W.T.astype(np.float16)  # [E, D]
        return np.ascontiguousarray(WT.reshape(EO, P, -1).transpose(1, 0, 2))

    consts = {
        "wq": prep_w(Wq),
        "wk": prep_w(Wk),
        "wv": prep_w(Wv),
        "bq": np.ascontiguousarray(bq.astype(np.float32)[:, None]),
        "bk": np.ascontiguousarray(bk.astype(np.float32)[:, None]),
        "bv": np.ascontiguousarray(bv.astype(np.float32)[:, None]),
        "mask": np.triu(np.ones((P, P), np.float16)),
        "ident": np.eye(P, dtype=np.float16),
    }
    return consts


def _prep_x(x):  # [S, E] f32 -> xT [E, S] fp16 -> [ei, j, eo, s_in_chunk]
    xT = x.astype(np.float16).T  # [E, S]
    x4 = xT.reshape(EO, P, NJ, NCH)  # [eo, ei, j, s]
    return np.ascontiguousarray(x4.transpose(1, 2, 0, 3))


def kernel(query, key_in, value, Wq, bq, Wk, bk, Wv, bv):
    global LAST_RESULT
    query = np.asarray(query, dtype=np.float32)
    key_in = np.asarray(key_in, dtype=np.float32)
    value = np.asarray(value, dtype=np.float32)
    consts = _prep_consts(
        np.asarray(Wq), np.asarray(bq), np.asarray(Wk),
        np.asarray(bk), np.asarray(Wv), np.asarray(bv),
    )
    in_maps = []
    for b in range(NCORES):
        m = dict(consts)
        m["qx"] = _prep_x(query[b])
        m["kx"] = _prep_x(key_in[b])
        m["vx"] = _prep_x(value[b])
        in_maps.append(m)

    nc = _get_nc()
    res = run_bass_kernel_spmd(nc, in_maps, core_ids=list(range(NCORES)))
    LAST_RESULT = res
    return np.stack([res.results[i]["out"] for i in range(NCORES)], axis=0)
